# revision 7
# baseline (speedup 1.0000x reference)
"""TRN2 Bass kernel for nn_Attention_76802605187492.

Math (B=64, T=512, H=1024, A=300):
  The aspect branch only adds a per-batch constant to the attention
  scores, which softmax cancels, so it does not affect the output.
  Per batch b:
    scores[t] = u . tanh(W_h hidden[b,t] + b_h)      u = w_w[0, :H]
    alpha     = softmax_t(scores)
    r         = sum_t alpha[t] hidden[b,t]
    p_b       = r @ W_p.T
    x_j       = hidden[j,-1] @ W_x.T                  (all j)
    out[b,j]  = tanh(p_b + x_j + (b_p + b_x))         -> [B, B, H]

Sharding: data-parallel over batch across 8 cores (8 batches each).

v2 design (vs 139us baseline):
  - big matmul, scores, r and p all in fp8 DoubleRow (2 k-rows/cycle).
    Scores pair m-tiles on the j dim (tanh writes fp8 tz directly);
    r contracts t via a p-major fp8 t-layout upload ht8 (hidden[b]
    .reshape(128,2,2,H), t = p*4 + tt2*2 + j), alpha transposed into
    masked am8 columns with 4 strided PE transposes per batch.
  - batches processed in waves of 2 (shared LDWEIGHTS per (m,kt2)),
    output pipelined in 2 groups of 4 batches so half the out DMA and
    the whole x2/p/G machinery for group A overlaps the main loop.
  - x2 term in bf16 hi@hi + lo@hi (hi@lo dropped, no wxT_lo upload);
    bias b_p+b_x rides the x2 psum via k=1 ones matmuls.
  - out written bf16 (host upcasts), halving the output DMA.
  - psum budget exactly 8 banks: z x4, s/tp x2, aux(x2/r/p/G) x2.
"""

import os
import sys

sys.path.insert(0, "/opt/trn_rl_repo")
sys.path.insert(0, "/opt/trn_rl_repo/concourse")

import numpy as np
import ml_dtypes

import concourse.bass as bass
import concourse.mybir as mybir
from concourse import tile
from concourse.bass_utils import run_bass_kernel_spmd

F32 = mybir.dt.float32
BF16 = mybir.dt.bfloat16
BF16_NP = ml_dtypes.bfloat16
TANH = mybir.ActivationFunctionType.Tanh
EXP = mybir.ActivationFunctionType.Exp
FP8 = mybir.dt.float8e4
FP8_NP = ml_dtypes.float8_e4m3
DR = mybir.MatmulPerfMode.DoubleRow
WSCALE = 16.0     # W_h pre-scale into fp8 range
USCALE = 128.0    # u pre-scale into fp8 range
ASCALE = 128.0    # alpha pre-scale (max alpha=1 -> 128 < 240 fp8 max)
RS = 32.0         # r pre-scale into fp8 range
WPS = 16.0        # W_p pre-scale into fp8 range

B, T, H = 64, 512, 1024
NCORES = 8
PB = B // NCORES          # batches per core = 8
KT = H // 128             # 8 k-tiles over h_in
MT = H // 128             # 8 m-tiles over h_out
KT2 = H // 256            # 4 double-row k-tiles
TT2 = T // 256            # 2 double-row t-tiles for r
NG = 2                    # output groups
GB = PB // NG             # batches per group = 4

_CACHE: dict = {}


def _build_nc() -> bass.Bass:
    nc = bass.Bass()

    xQ8 = nc.declare_dram_parameter("xQ8", [PB, 128, KT2 * 2 * T], FP8, isOutput=False)
    whQ8 = nc.declare_dram_parameter(
        "whQ8", [MT, 128, KT2 * 2 * 128], FP8, isOutput=False
    )
    ht8 = nc.declare_dram_parameter("ht8", [PB, 128, TT2 * 2 * H], FP8, isOutput=False)
    bh = nc.declare_dram_parameter("bh", [128, MT], F32, isOutput=False)
    uu = nc.declare_dram_parameter("u8", [128, 2 * 16], FP8, isOutput=False)
    wpT8 = nc.declare_dram_parameter(
        "wpT8", [128, KT2 * 2 * H], FP8, isOutput=False
    )
    wxh = nc.declare_dram_parameter("wxT_hi", [H, H], BF16, isOutput=False)
    hlh = nc.declare_dram_parameter("hlastT_hi", [H, B], BF16, isOutput=False)
    hll = nc.declare_dram_parameter("hlastT_lo", [H, B], BF16, isOutput=False)
    selg = nc.declare_dram_parameter("selg", [GB, 2 * 128], BF16, isOutput=False)
    bpx = nc.declare_dram_parameter("bpx", [1, 2 * H], BF16, isOutput=False)
    ones = nc.declare_dram_parameter("ones", [1, B], BF16, isOutput=False)
    ident = nc.declare_dram_parameter("ident", [GB, GB], BF16, isOutput=False)
    out = nc.declare_dram_parameter("out", [PB, B, H], BF16, isOutput=True)

    with tile.TileContext(nc) as tc:
        with (
            tc.tile_pool(name="const", bufs=1) as cp,
            tc.tile_pool(name="xchunk", bufs=1) as xp,
            tc.tile_pool(name="hts", bufs=1) as hp,
            tc.tile_pool(name="tz", bufs=6) as tzp,
            tc.tile_pool(name="small", bufs=1) as sp,
            tc.tile_pool(name="sc", bufs=2) as scp,
            tc.tile_pool(name="outp", bufs=4) as op_,
            tc.tile_pool(name="zps", bufs=4, space=bass.MemorySpace.PSUM) as zp,
            tc.tile_pool(name="sps", bufs=2, space=bass.MemorySpace.PSUM) as spp,
            tc.tile_pool(name="aux", bufs=2, space=bass.MemorySpace.PSUM) as ap,
        ):
            # ---- tiny constants first (sync queue) ----
            bh_sb = cp.tile([128, MT], F32)
            nc.sync.dma_start(bh_sb[:], bh[:])
            u8_sb = cp.tile([128, 2, 16], FP8)
            nc.sync.dma_start(u8_sb[:], uu[:].rearrange("p (j q) -> p j q", j=2))
            id_sb = cp.tile([GB, GB], BF16)
            nc.sync.dma_start(id_sb[:], ident[:])
            selg_sb = cp.tile([GB, 2, 128], BF16)
            nc.sync.dma_start(selg_sb[:], selg[:].rearrange("g (q m) -> g q m", q=2))
            bpx_sb = cp.tile([1, 2 * H], BF16)
            nc.sync.dma_start(bpx_sb[:], bpx[:])
            ones_sb = cp.tile([1, B], BF16)
            nc.sync.dma_start(ones_sb[:], ones[:])

            # big-mm weights: m=0 first so the first matmul can start asap
            wm_sb = []

            def _load_wm(m):
                wm = cp.tile([128, KT2, 2, 128], FP8, name=f"wm{m}")
                nc.sync.dma_start(
                    wm[:], whQ8[m].rearrange("p (kt j o) -> p kt j o", j=2, o=128)
                )
                return wm

            wm_sb.append(_load_wm(0))

            # xc chunks: kt2-sliced DMAs so the first matmul waits on 1/4 chunk
            xc_sb = []

            def _load_xc(b):
                xc = xp.tile([128, KT2, 2, T], FP8, name=f"xc{b}")
                for kt in range(KT2):
                    nc.sync.dma_start(
                        xc[:, kt],
                        xQ8[b]
                        .rearrange("p (kt j n) -> p kt j n", j=2, n=T)[:, kt],
                    )
                return xc

            xc_sb.append(_load_xc(0))
            xc_sb.append(_load_xc(1))
            for m in range(1, MT):
                wm_sb.append(_load_wm(m))
            xc_sb.append(_load_xc(2))
            xc_sb.append(_load_xc(3))
            # W_p after the first 4 xc chunks (needed at boundary A)
            wpT_sb = cp.tile([128, KT2, 2, H], FP8)
            nc.sync.dma_start(
                wpT_sb[:], wpT8[:].rearrange("p (kt j n) -> p kt j n", j=2, n=H)
            )
            for b2 in range(4, PB):
                xc_sb.append(_load_xc(b2))

            # gpsimd queue: ht8 group A, then x2 weights, then ht8 group B
            ht_sb = [None] * PB

            def _load_ht(b):
                ht = hp.tile([128, TT2, 2, H], FP8, name=f"ht{b}")
                nc.gpsimd.dma_start(
                    ht[:], ht8[b].rearrange("p (tt j h) -> p tt j h", j=2, h=H)
                )
                return ht

            for b in range(GB):
                ht_sb[b] = _load_ht(b)
            wxh_sb = cp.tile([128, KT, H], BF16)
            nc.gpsimd.dma_start(
                wxh_sb[:], wxh[:].rearrange("(kt p) n -> p kt n", p=128)
            )
            hlh_sb = cp.tile([128, KT, B], BF16)
            nc.gpsimd.dma_start(
                hlh_sb[:], hlh[:].rearrange("(kt p) j -> p kt j", p=128)
            )
            hll_sb = cp.tile([128, KT, B], BF16)
            nc.gpsimd.dma_start(
                hll_sb[:], hll[:].rearrange("(kt p) j -> p kt j", p=128)
            )
            for b in range(GB, PB):
                ht_sb[b] = _load_ht(b)

            # masked alphaT tiles per group: [128, tt2, j, b_local, col]
            am8 = [None, None]
            x2_sb = sp.tile([128, H], F32)

            def softmax_alpha(b):
                """s_ps[b%2] -> alpha8 -> 4 strided transposes into am8."""
                g, bl = b // GB, b % GB
                e_b = scp.tile([1, T], F32, tag="eb")
                nc.scalar.activation(e_b[:1], s_ps[b % 2][:1], EXP, scale=1.0 / USCALE)
                esum = scp.tile([1, 1], F32, tag="es")
                nc.vector.reduce_sum(esum[:1], e_b[:1], axis=mybir.AxisListType.X)
                einv = scp.tile([1, 1], F32, tag="ei")
                nc.vector.reciprocal(einv[:1], esum[:1])
                a8 = scp.tile([1, T], BF16, tag="ab")
                nc.vector.tensor_scalar(
                    a8[:1],
                    e_b[:1],
                    einv[:1, :1],
                    ASCALE,
                    mybir.AluOpType.mult,
                    mybir.AluOpType.mult,
                )
                # t = (tt2*2 + jj)*128 + p: contiguous chunk -> column bl
                for tt in range(TT2):
                    for jj in range(2):
                        c = tt * 2 + jj
                        tp_ps = spp.tile([128, 1], BF16, tag="s", name="tp")
                        nc.tensor.transpose(
                            tp_ps[:, :1],
                            a8[:1, c * 128 : (c + 1) * 128],
                            id_sb[:1, :1],
                        )
                        nc.vector.tensor_scalar_mul(
                            am8[g][:, tt, jj, bl, bl : bl + 1], tp_ps[:, :1], 1.0
                        )

            def boundary(g):
                """burst r for group g, (x2 if g==0), p, G tiles + out DMA."""
                # r: per hc sequentially so only one aux pair bank is live
                p_sb = sp.tile([GB, H], BF16, tag="psb", name=f"p{g}", bufs=2)
                rT8 = sp.tile([128, KT2, 2, 16], FP8, tag="rT", name=f"rT{g}", bufs=2)
                for hc in range(2):
                    r_ps = ap.tile([GB, 512], F32, tag="aux", name=f"r{g}{hc}")
                    n = 0
                    for bl in range(GB):
                        for tt in range(TT2):
                            nc.tensor.matmul(
                                r_ps[:],
                                am8[g][:, tt, :, bl, :],
                                ht_sb[g * GB + bl][:, tt, :, hc * 512 : (hc + 1) * 512],
                                start=(n == 0),
                                stop=(n == 2 * GB - 1),
                                perf_mode=DR,
                            )
                            n += 1
                    rflat8 = sp.tile([GB, 512], BF16, tag="rf", name="rflat8", bufs=4)
                    nc.vector.tensor_scalar_mul(rflat8[:GB], r_ps[:], RS / ASCALE)
                    # rT: h = (kt2*2+j)*128 + p = (hc*4 + ktl)*128 + p
                    for ktl in range(4):
                        tp_ps = spp.tile([128, GB], BF16, tag="s", name="tpr")
                        nc.tensor.transpose(
                            tp_ps[:, :GB],
                            rflat8[:GB, ktl * 128 : (ktl + 1) * 128],
                            id_sb[:GB, :GB],
                        )
                        kk = hc * 4 + ktl
                        nc.vector.tensor_scalar_mul(
                            rT8[:, kk // 2, kk % 2, :GB], tp_ps[:, :GB], 1.0
                        )
                if g == 0:
                    # x2 = hlast @ W_x.T + (b_p + b_x), bf16 hi@hi + lo@hi
                    for hc in range(2):
                        x_ps = ap.tile([B, 512], F32, tag="aux", name=f"x{hc}")
                        n = 0
                        nmm = 2 * KT + 2
                        for lh in (hlh_sb, hll_sb):
                            for kt in range(KT):
                                nc.tensor.matmul(
                                    x_ps[:],
                                    lh[:, kt, :],
                                    wxh_sb[:, kt, hc * 512 : (hc + 1) * 512],
                                    start=(n == 0),
                                    stop=(n == nmm - 1),
                                )
                                n += 1
                        for row in range(2):
                            nc.tensor.matmul(
                                x_ps[:],
                                ones_sb[:1, :],
                                bpx_sb[:1, row * H + hc * 512 : row * H + (hc + 1) * 512],
                                start=(n == 0),
                                stop=(n == nmm - 1),
                            )
                            n += 1
                        nc.vector.tensor_scalar_mul(
                            x2_sb[:B, hc * 512 : (hc + 1) * 512], x_ps[:], 1.0
                        )
                        nc.vector.tensor_scalar_mul(
                            x2_sb[B:, hc * 512 : (hc + 1) * 512], x_ps[:], 1.0
                        )
                # p = r @ W_p.T in fp8 DR
                for hc in range(2):
                    p_ps = ap.tile([GB, 512], F32, tag="aux", name=f"pp{g}{hc}")
                    for kt in range(KT2):
                        nc.tensor.matmul(
                            p_ps[:],
                            rT8[:, kt, :, :GB],
                            wpT_sb[:, kt, :, hc * 512 : (hc + 1) * 512],
                            start=(kt == 0),
                            stop=(kt == KT2 - 1),
                            perf_mode=DR,
                        )
                    nc.vector.tensor_scalar_mul(
                        p_sb[:GB, hc * 512 : (hc + 1) * 512],
                        p_ps[:],
                        1.0 / (RS * WPS),
                    )
                # G: out rows for this group's 4 batches
                for q in range(2):
                    for hc in range(2):
                        o_ps = ap.tile([128, 512], F32, tag="aux", name=f"o{g}{q}{hc}")
                        nc.tensor.matmul(
                            o_ps[:],
                            selg_sb[:, q, :],
                            p_sb[:GB, hc * 512 : (hc + 1) * 512],
                            start=True,
                            stop=True,
                        )
                        o_sb = op_.tile([128, 512], F32, tag="oadd")
                        nc.vector.tensor_tensor(
                            o_sb[:],
                            o_ps[:],
                            x2_sb[:, hc * 512 : (hc + 1) * 512],
                            mybir.AluOpType.add,
                        )
                        o2 = op_.tile([128, 512], BF16, tag="otanh")
                        nc.scalar.activation(o2[:], o_sb[:], TANH)
                        i0 = g * GB + 2 * q
                        nc.scalar.dma_start(
                            out[i0 : i0 + 2, :, hc * 512 : (hc + 1) * 512].rearrange(
                                "i j h -> (i j) h"
                            ),
                            o2[:],
                        )

            # ---- main loop: waves of 2 batches ----
            s_ps = [None, None]
            for w in range(PB // 2):
                b0 = 2 * w
                g = b0 // GB
                if b0 % GB == 0:
                    am = sp.tile(
                        [128, TT2, 2, GB, GB], FP8, tag=f"am{g}", name=f"am8_{g}"
                    )
                    nc.vector.memset(am[:], 0.0)
                    am8[g] = am
                s_ps[0] = spp.tile([1, T], F32, tag="s", name=f"s{b0}")
                s_ps[1] = spp.tile([1, T], F32, tag="s", name=f"s{b0 + 1}")
                tz8 = [None, None]
                for m in range(MT):
                    z_ps = [
                        zp.tile([128, T], F32, tag="z", name=f"z{b0 + b2}_{m}")
                        for b2 in range(2)
                    ]
                    for kt in range(KT2):
                        for b2 in range(2):
                            nc.tensor.matmul(
                                z_ps[b2][:],
                                wm_sb[m][:, kt, :, :],
                                xc_sb[b0 + b2][:, kt, :, :],
                                start=(kt == 0),
                                stop=(kt == KT2 - 1),
                                perf_mode=DR,
                            )
                    q, jj = m // 2, m % 2
                    for b2 in range(2):
                        if jj == 0:
                            tz8[b2] = tzp.tile(
                                [128, 2, T], FP8, tag="tz8", name=f"tz{b0+b2}_{q}"
                            )
                        nc.scalar.activation(
                            tz8[b2][:, jj, :],
                            z_ps[b2][:],
                            TANH,
                            bias=bh_sb[:, m : m + 1],
                            scale=1.0 / WSCALE,
                        )
                    if jj == 1:
                        for b2 in range(2):
                            nc.tensor.matmul(
                                s_ps[b2][:1, :],
                                u8_sb[:, :, q : q + 1],
                                tz8[b2][:],
                                start=(q == 0),
                                stop=(q == KT2 - 1),
                                perf_mode=DR,
                            )
                for b2 in range(2):
                    softmax_alpha(b0 + b2)
                if b0 % GB == 2:
                    boundary(g)
    _split_excess_waits(nc)
    return nc


def _split_excess_waits(nc: bass.Bass, max_waits: int = 1) -> None:
    """Walrus's per-instruction sync-wait slots are limited; move excess
    on_wait entries onto wait-only NoOps inserted just before the
    instruction (same engine, so ordering is preserved)."""
    for fn in nc.m.functions:
        for blk in fn.blocks:
            new = []
            for inst in blk.instructions:
                si = inst.sync_info
                waits = list(si.on_wait) if si is not None and si.on_wait else []
                if len(waits) > max_waits:
                    extra, keep = waits[:-max_waits], waits[-max_waits:]
                    for ci in range(0, len(extra), max_waits):
                        nop = mybir.InstNoOp(
                            name=f"{inst.name}-wsplit{ci}", ins=[], outs=[]
                        )
                        nop.engine = inst.engine
                        nop.sync_info = mybir.SyncInfo(
                            on_wait=extra[ci : ci + max_waits], on_update=[]
                        )
                        new.append(nop)
                    inst.sync_info = mybir.SyncInfo(
                        on_wait=keep, on_update=list(si.on_update or [])
                    )
                new.append(inst)
            blk.instructions[:] = new


def _split_bf16(a: np.ndarray) -> tuple[np.ndarray, np.ndarray]:
    hi = a.astype(BF16_NP)
    lo = (a - hi.astype(np.float32)).astype(BF16_NP)
    return hi, lo


def _pad_u8(w_w: np.ndarray) -> np.ndarray:
    u = np.zeros((128, 2, 16), np.float32)
    u[:, :, :KT2] = (w_w[0, :H] * USCALE).reshape(KT2, 2, 128).transpose(2, 1, 0)
    return np.ascontiguousarray(u.reshape(128, 2 * 16)).astype(FP8_NP)


def _host_prep(inputs: dict) -> list[dict]:
    hidden = np.asarray(inputs["hidden"], np.float32)
    W_h = np.asarray(inputs["W_h"], np.float32)
    b_h = np.asarray(inputs["b_h"], np.float32)
    w_w = np.asarray(inputs["w_w"], np.float32)
    W_p = np.asarray(inputs["W_p"], np.float32)
    b_p = np.asarray(inputs["b_p"], np.float32)
    W_x = np.asarray(inputs["W_x"], np.float32)
    b_x = np.asarray(inputs["b_x"], np.float32)

    # G selector: group-local row i = 2q + m//64
    selgm = np.zeros((GB, 2, 128), np.float32)
    for q in range(2):
        for m in range(128):
            selgm[2 * q + m // 64, q, m] = 1.0

    wxT = np.ascontiguousarray(W_x.T)
    wx_hi, _ = _split_bf16(wxT)
    hlT = np.ascontiguousarray(hidden[:, -1, :].T)
    hl_hi, hl_lo = _split_bf16(hlT)
    bpx_hi, bpx_lo = _split_bf16((b_p + b_x).reshape(1, H))

    shared = {
        "whQ8": np.ascontiguousarray(
            (W_h.T * WSCALE)
            .reshape(KT2, 128, 2, MT, 128)
            .transpose(3, 1, 0, 2, 4)
            .reshape(MT, 128, KT2 * 2 * 128)
        ).astype(FP8_NP),
        "bh": np.ascontiguousarray(b_h.reshape(MT, 128).T),
        # u8[p, j, q] = u[(2q+j)*128 + p] * USCALE, q padded to 16
        "u8": _pad_u8(w_w),
        # wpT8[p, kt2, j, n] = W_p.T[(kt2*2+j)*128 + p, n] * WPS
        "wpT8": np.ascontiguousarray(
            (W_p.T * WPS).reshape(KT2, 2, 128, H).transpose(2, 0, 1, 3).reshape(
                128, KT2 * 2 * H
            )
        ).astype(FP8_NP),
        "wxT_hi": wx_hi,
        "hlastT_hi": hl_hi,
        "hlastT_lo": hl_lo,
        "selg": selgm.reshape(GB, 2 * 128).astype(BF16_NP),
        "bpx": np.concatenate([bpx_hi, bpx_lo], axis=1),
        "ones": np.ones((1, B), BF16_NP),
        "ident": np.eye(GB, dtype=np.float32).astype(BF16_NP),
    }

    in_maps = []
    for c in range(NCORES):
        slab = hidden[c * PB : (c + 1) * PB]          # [PB, T, H]
        m = dict(shared)
        m["xQ8"] = np.ascontiguousarray(
            slab.reshape(PB, T, KT2, 128, 2)
            .transpose(0, 3, 2, 4, 1)
            .reshape(PB, 128, KT2 * 2 * T)
        ).astype(FP8_NP)
        # ht8[b][p, tt2, j, h] = hidden[b, t = (tt2*2+j)*128 + p, h]
        m["ht8"] = np.ascontiguousarray(
            slab.reshape(PB, TT2, 2, 128, H)
            .transpose(0, 3, 1, 2, 4)
            .reshape(PB, 128, TT2 * 2 * H)
        ).astype(FP8_NP)
        in_maps.append(m)
    return in_maps


def _ensure_ntff_hook() -> None:
    """The agent image's antenv lacks axon_hooks; register a shim module
    wired to the libaxon NTFF profile hook so trace=True works."""
    try:
        from antenv.axon_hooks import get_axon_ntff_profile_hook  # noqa: F401
        return
    except ImportError:
        pass
    import types
    import antenv
    from trn_agent_boot.trn_boot import _ntff_profile_via_ctypes

    mod = types.ModuleType("antenv.axon_hooks")
    holder = {"hook": _ntff_profile_via_ctypes("/opt/axon/libaxon_pjrt.so")}
    mod.get_axon_ntff_profile_hook = lambda: holder["hook"]
    mod.set_axon_ntff_profile_hook = lambda h: holder.__setitem__("hook", h)
    sys.modules["antenv.axon_hooks"] = mod
    antenv.axon_hooks = mod


def run(inputs: dict, trace: bool = False, **kw):
    if trace:
        _ensure_ntff_hook()
    if "nc" not in _CACHE:
        _CACHE["nc"] = _build_nc()
    nc = _CACHE["nc"]
    in_maps = _host_prep(inputs)
    res = run_bass_kernel_spmd(nc, in_maps, list(range(NCORES)), trace=trace, **kw)
    out = np.empty((B, B, H), np.float32)
    for c in range(NCORES):
        out[c * PB : (c + 1) * PB] = np.asarray(
            res.results[c]["out"], np.float32
        )
    return out, res


def kernel(**inputs) -> np.ndarray:
    out, _ = run(inputs)
    return out


# revision 8
# speedup vs baseline: 1.0385x; 1.0385x over previous
"""TRN2 Bass kernel for nn_Attention_76802605187492.

Math (B=64, T=512, H=1024, A=300):
  The aspect branch only adds a per-batch constant to the attention
  scores, which softmax cancels, so it does not affect the output.
  Per batch b:
    scores[t] = u . tanh(W_h hidden[b,t] + b_h)      u = w_w[0, :H]
    alpha     = softmax_t(scores)
    r         = sum_t alpha[t] hidden[b,t]
    p_b       = r @ W_p.T
    x_j       = hidden[j,-1] @ W_x.T                  (all j)
    out[b,j]  = tanh(p_b + x_j + (b_p + b_x))         -> [B, B, H]

Sharding: data-parallel over batch across 8 cores (8 batches each).

v3 design (vs 139us baseline):
  - big matmul, scores, r and p all in fp8 DoubleRow (2 k-rows/cycle).
    Scores pair m-tiles on the j dim (tanh writes fp8 tz directly);
    r contracts t via an fp8 t-layout upload ht8 (t = (tt2*2+j)*128+p),
    alpha transposed into masked am8 columns, 4 chunks per batch.
    DR ldweights requires the j-plane stride to be >= 16 elements
    (u8/rT8 padded accordingly).
  - batches processed in waves of 2; ALL input DMAs ride one sync-queue
    in consumption order (consts, wm0, xc0-1, wm1-7, wx/hl, xc2-5,
    ht0-3, xc6-7, wpT8, ht4-7); outputs go out on the scalar queue.
  - x2 (hlast @ W_x.T, bf16 hi@hi + lo@hi; bias rides via k=1 ones
    matmuls) is issued right after wave 0 where the PE is DMA-starved.
  - output pipelined in 3 groups (batches 0-3, 4-5, 6-7): each group's
    r-burst/rT/p/G/out-DMA runs right after its last wave, so only the
    last 2 batches' output (0.26MB) drains at the end.
  - softmax exp uses accum_out to fuse the row-sum.
  - psum: z x4 (2KB slots, also used by boundary transposes), s x2,
    aux x2 = exactly 8 banks.
"""

import os
import sys

sys.path.insert(0, "/opt/trn_rl_repo")
sys.path.insert(0, "/opt/trn_rl_repo/concourse")

import numpy as np
import ml_dtypes

import concourse.bass as bass
import concourse.mybir as mybir
from concourse import tile
from concourse.bass_utils import run_bass_kernel_spmd

F32 = mybir.dt.float32
BF16 = mybir.dt.bfloat16
BF16_NP = ml_dtypes.bfloat16
TANH = mybir.ActivationFunctionType.Tanh
EXP = mybir.ActivationFunctionType.Exp
FP8 = mybir.dt.float8e4
FP8_NP = ml_dtypes.float8_e4m3
DR = mybir.MatmulPerfMode.DoubleRow
WSCALE = 16.0     # W_h pre-scale into fp8 range
USCALE = 128.0    # u pre-scale into fp8 range
ASCALE = 128.0    # alpha pre-scale (max alpha=1 -> 128 < 240 fp8 max)
RS = 32.0         # r pre-scale into fp8 range
WPS = 16.0        # W_p pre-scale into fp8 range

B, T, H = 64, 512, 1024
NCORES = 8
PB = B // NCORES          # batches per core = 8
KT = H // 128             # 8 k-tiles over h_in
MT = H // 128             # 8 m-tiles over h_out
KT2 = H // 256            # 4 double-row k-tiles
TT2 = T // 256            # 2 double-row t-tiles for r
GROUPS = [(0, 4), (4, 2), (6, 2)]   # (first batch, size) output groups

_CACHE: dict = {}


def _build_nc() -> bass.Bass:
    nc = bass.Bass()

    xQ8 = nc.declare_dram_parameter("xQ8", [PB, 128, KT2 * 2 * T], FP8, isOutput=False)
    whQ8 = nc.declare_dram_parameter(
        "whQ8", [MT, 128, KT2 * 2 * 128], FP8, isOutput=False
    )
    ht8 = nc.declare_dram_parameter("ht8", [PB, 128, TT2 * 2 * H], FP8, isOutput=False)
    bh = nc.declare_dram_parameter("bh", [128, MT], F32, isOutput=False)
    uu = nc.declare_dram_parameter("u8", [128, 2 * 16], FP8, isOutput=False)
    wpT8 = nc.declare_dram_parameter("wpT8", [128, KT2 * 2 * H], FP8, isOutput=False)
    wxh = nc.declare_dram_parameter("wxT_hi", [H, H], BF16, isOutput=False)
    hlh = nc.declare_dram_parameter("hlastT_hi", [H, B], BF16, isOutput=False)
    hll = nc.declare_dram_parameter("hlastT_lo", [H, B], BF16, isOutput=False)
    selg = nc.declare_dram_parameter("selg", [4, 2 * 128], BF16, isOutput=False)
    bpx = nc.declare_dram_parameter("bpx", [1, 2 * H], BF16, isOutput=False)
    ones = nc.declare_dram_parameter("ones", [1, B], BF16, isOutput=False)
    ident = nc.declare_dram_parameter("ident", [4, 4], BF16, isOutput=False)
    out = nc.declare_dram_parameter("out", [PB, B, H], BF16, isOutput=True)

    with tile.TileContext(nc) as tc:
        with (
            tc.tile_pool(name="const", bufs=1) as cp,
            tc.tile_pool(name="xchunk", bufs=1) as xp,
            tc.tile_pool(name="hts", bufs=1) as hp,
            tc.tile_pool(name="tz", bufs=6) as tzp,
            tc.tile_pool(name="small", bufs=1) as sp,
            tc.tile_pool(name="sc", bufs=2) as scp,
            tc.tile_pool(name="outp", bufs=4) as op_,
            tc.tile_pool(name="zps", bufs=4, space=bass.MemorySpace.PSUM) as zp,
            tc.tile_pool(name="sps", bufs=2, space=bass.MemorySpace.PSUM) as spp,
            tc.tile_pool(name="aux", bufs=2, space=bass.MemorySpace.PSUM) as ap,
        ):
            # ---- input DMAs, all on the sync queue in consumption order ----
            bh_sb = cp.tile([128, MT], F32)
            nc.sync.dma_start(bh_sb[:], bh[:])
            u8_sb = cp.tile([128, 2, 16], FP8)
            nc.sync.dma_start(u8_sb[:], uu[:].rearrange("p (j q) -> p j q", j=2))
            id_sb = cp.tile([4, 4], BF16)
            nc.sync.dma_start(id_sb[:], ident[:])
            selg_sb = cp.tile([4, 2, 128], BF16)
            nc.sync.dma_start(selg_sb[:], selg[:].rearrange("g (q m) -> g q m", q=2))
            bpx_sb = cp.tile([1, 2 * H], BF16)
            nc.sync.dma_start(bpx_sb[:], bpx[:])
            ones_sb = cp.tile([1, B], BF16)
            nc.sync.dma_start(ones_sb[:], ones[:])

            wm_sb = []

            def _load_wm(m):
                wm = cp.tile([128, KT2, 2, 128], FP8, name=f"wm{m}")
                nc.sync.dma_start(
                    wm[:], whQ8[m].rearrange("p (kt j o) -> p kt j o", j=2, o=128)
                )
                return wm

            xc_sb = []

            def _load_xc(b):
                xc = xp.tile([128, KT2, 2, T], FP8, name=f"xc{b}")
                for kt in range(KT2):
                    nc.sync.dma_start(
                        xc[:, kt],
                        xQ8[b].rearrange("p (kt j n) -> p kt j n", j=2, n=T)[:, kt],
                    )
                return xc

            ht_sb = [None] * PB

            def _load_ht(b):
                ht = hp.tile([128, TT2, 2, H], FP8, name=f"ht{b}")
                nc.sync.dma_start(
                    ht[:], ht8[b].rearrange("p (tt j h) -> p tt j h", j=2, h=H)
                )
                return ht

            wm_sb.append(_load_wm(0))
            xc_sb.append(_load_xc(0))
            xc_sb.append(_load_xc(1))
            for m in range(1, MT):
                wm_sb.append(_load_wm(m))
            wxh_sb = cp.tile([128, KT, H], BF16)
            nc.sync.dma_start(wxh_sb[:], wxh[:].rearrange("(kt p) n -> p kt n", p=128))
            hlh_sb = cp.tile([128, KT, B], BF16)
            nc.sync.dma_start(hlh_sb[:], hlh[:].rearrange("(kt p) j -> p kt j", p=128))
            hll_sb = cp.tile([128, KT, B], BF16)
            nc.sync.dma_start(hll_sb[:], hll[:].rearrange("(kt p) j -> p kt j", p=128))
            for b in (2, 3, 4, 5):
                xc_sb.append(_load_xc(b))
            for b in range(4):
                ht_sb[b] = _load_ht(b)
            for b in (6, 7):
                xc_sb.append(_load_xc(b))
            wpT_sb = cp.tile([128, KT2, 2, H], FP8)
            nc.sync.dma_start(
                wpT_sb[:], wpT8[:].rearrange("p (kt j n) -> p kt j n", j=2, n=H)
            )
            for b in range(4, PB):
                ht_sb[b] = _load_ht(b)

            am8 = [None] * len(GROUPS)
            x2_sb = sp.tile([128, H], F32)
            s_ps = [None, None]

            def softmax_alpha(b, g, bl):
                """s_ps[b%2] -> alpha -> 4 chunk transposes into am8[g]."""
                e_b = scp.tile([1, T], F32, tag="eb")
                esum = scp.tile([1, 1], F32, tag="es")
                nc.scalar.activation(
                    e_b[:1], s_ps[b % 2][:1], EXP, scale=1.0 / USCALE,
                    accum_out=esum[:1],
                )
                einv = scp.tile([1, 1], F32, tag="ei")
                nc.vector.reciprocal(einv[:1], esum[:1])
                a8 = scp.tile([1, T], BF16, tag="ab")
                nc.vector.tensor_scalar(
                    a8[:1],
                    e_b[:1],
                    einv[:1, :1],
                    ASCALE,
                    mybir.AluOpType.mult,
                    mybir.AluOpType.mult,
                )
                # t = (tt2*2 + jj)*128 + p: chunk c -> column bl of block bl
                for c in range(4):
                    tp_ps = zp.tile([128, 1], BF16, tag="z", name="tp")
                    nc.tensor.transpose(
                        tp_ps[:, :1], a8[:1, c * 128 : (c + 1) * 128], id_sb[:1, :1]
                    )
                    nc.vector.tensor_scalar_mul(
                        am8[g][:, c // 2, c % 2, bl, bl : bl + 1], tp_ps[:, :1], 1.0
                    )

            def emit_x2():
                """x2 = hlast @ W_x.T + (b_p + b_x), bf16 hi@hi + lo@hi."""
                for hc in range(2):
                    x_ps = ap.tile([B, 512], F32, tag="aux", name=f"x{hc}")
                    n = 0
                    nmm = 2 * KT + 2
                    for lh in (hlh_sb, hll_sb):
                        for kt in range(KT):
                            nc.tensor.matmul(
                                x_ps[:],
                                lh[:, kt, :],
                                wxh_sb[:, kt, hc * 512 : (hc + 1) * 512],
                                start=(n == 0),
                                stop=(n == nmm - 1),
                            )
                            n += 1
                    for row in range(2):
                        nc.tensor.matmul(
                            x_ps[:],
                            ones_sb[:1, :],
                            bpx_sb[:1, row * H + hc * 512 : row * H + (hc + 1) * 512],
                            start=(n == 0),
                            stop=(n == nmm - 1),
                        )
                        n += 1
                    nc.vector.tensor_scalar_mul(
                        x2_sb[:B, hc * 512 : (hc + 1) * 512], x_ps[:], 1.0
                    )
                    nc.vector.tensor_scalar_mul(
                        x2_sb[B:, hc * 512 : (hc + 1) * 512], x_ps[:], 1.0
                    )

            def boundary(g):
                """r-burst for group g, rT, p, G tiles + out DMA."""
                start, size = GROUPS[g]
                p_sb = sp.tile([4, H], BF16, tag="psb", name=f"p{g}", bufs=2)
                rT8 = sp.tile([128, KT2, 2, 16], FP8, tag="rT", name=f"rT{g}", bufs=2)
                rfl = [None, None]
                # both hc r-bursts first (2 aux banks), then rflat, then rT
                for hc in range(2):
                    r_ps = ap.tile([4, 512], F32, tag="aux", name=f"r{g}{hc}")
                    n = 0
                    for bl in range(size):
                        for tt in range(TT2):
                            nc.tensor.matmul(
                                r_ps[:size],
                                am8[g][:, tt, :, bl, :size],
                                ht_sb[start + bl][:, tt, :, hc * 512 : (hc + 1) * 512],
                                start=(n == 0),
                                stop=(n == 2 * size - 1),
                                perf_mode=DR,
                            )
                            n += 1
                    rflat = sp.tile([4, 512], BF16, tag="rf", name="rflat", bufs=4)
                    nc.vector.tensor_scalar_mul(rflat[:size], r_ps[:size], RS / ASCALE)
                    rfl[hc] = rflat
                # rT: h = (kt2*2+j)*128 + p = (hc*4 + ktl)*128 + p
                tps = []
                for hc in range(2):
                    for ktl in range(4):
                        tp_ps = zp.tile([128, 4], BF16, tag="z", name="tpr")
                        nc.tensor.transpose(
                            tp_ps[:, :size],
                            rfl[hc][:size, ktl * 128 : (ktl + 1) * 128],
                            id_sb[:size, :size],
                        )
                        tps.append((hc * 4 + ktl, tp_ps))
                for kk, tp_ps in tps:
                    nc.vector.tensor_scalar_mul(
                        rT8[:, kk // 2, kk % 2, :size], tp_ps[:, :size], 1.0
                    )
                # p = r @ W_p.T in fp8 DR
                for hc in range(2):
                    p_ps = ap.tile([4, 512], F32, tag="aux", name=f"pp{g}{hc}")
                    for kt in range(KT2):
                        nc.tensor.matmul(
                            p_ps[:size],
                            rT8[:, kt, :, :size],
                            wpT_sb[:, kt, :, hc * 512 : (hc + 1) * 512],
                            start=(kt == 0),
                            stop=(kt == KT2 - 1),
                            perf_mode=DR,
                        )
                    nc.vector.tensor_scalar_mul(
                        p_sb[:size, hc * 512 : (hc + 1) * 512],
                        p_ps[:size],
                        1.0 / (RS * WPS),
                    )
                # G: out rows for this group's batches (2 rows per tile)
                for q in range(size // 2):
                    for hc in range(2):
                        o_ps = ap.tile([128, 512], F32, tag="aux", name=f"o{g}{q}{hc}")
                        nc.tensor.matmul(
                            o_ps[:],
                            selg_sb[:size, q, :],
                            p_sb[:size, hc * 512 : (hc + 1) * 512],
                            start=True,
                            stop=True,
                        )
                        o_sb = op_.tile([128, 512], F32, tag="oadd")
                        nc.vector.tensor_tensor(
                            o_sb[:],
                            o_ps[:],
                            x2_sb[:, hc * 512 : (hc + 1) * 512],
                            mybir.AluOpType.add,
                        )
                        o2 = op_.tile([128, 512], BF16, tag="otanh")
                        nc.scalar.activation(o2[:], o_sb[:], TANH)
                        i0 = start + 2 * q
                        nc.scalar.dma_start(
                            out[i0 : i0 + 2, :, hc * 512 : (hc + 1) * 512].rearrange(
                                "i j h -> (i j) h"
                            ),
                            o2[:],
                        )

            # ---- main loop: waves of 2 batches ----
            for w in range(PB // 2):
                b0 = 2 * w
                g = next(i for i, (s, n) in enumerate(GROUPS) if s <= b0 < s + n)
                gstart, gsize = GROUPS[g]
                if b0 == gstart:
                    am = sp.tile(
                        [128, TT2, 2, 4, 4], FP8, tag=f"am{g}", name=f"am8_{g}"
                    )
                    nc.vector.memset(am[:], 0.0)
                    am8[g] = am
                s_ps[0] = spp.tile([1, T], F32, tag="s", name=f"s{b0}")
                s_ps[1] = spp.tile([1, T], F32, tag="s", name=f"s{b0 + 1}")
                tz8 = [None, None]
                for m in range(MT):
                    z_ps = [
                        zp.tile([128, T], F32, tag="z", name=f"z{b0 + b2}_{m}")
                        for b2 in range(2)
                    ]
                    for kt in range(KT2):
                        for b2 in range(2):
                            nc.tensor.matmul(
                                z_ps[b2][:],
                                wm_sb[m][:, kt, :, :],
                                xc_sb[b0 + b2][:, kt, :, :],
                                start=(kt == 0),
                                stop=(kt == KT2 - 1),
                                perf_mode=DR,
                            )
                    q, jj = m // 2, m % 2
                    for b2 in range(2):
                        if jj == 0:
                            tz8[b2] = tzp.tile(
                                [128, 2, T], FP8, tag="tz8", name=f"tz{b0+b2}_{q}"
                            )
                        nc.scalar.activation(
                            tz8[b2][:, jj, :],
                            z_ps[b2][:],
                            TANH,
                            bias=bh_sb[:, m : m + 1],
                            scale=1.0 / WSCALE,
                        )
                    if jj == 1:
                        for b2 in range(2):
                            nc.tensor.matmul(
                                s_ps[b2][:1, :],
                                u8_sb[:, :, q : q + 1],
                                tz8[b2][:],
                                start=(q == 0),
                                stop=(q == KT2 - 1),
                                perf_mode=DR,
                            )
                for b2 in range(2):
                    b = b0 + b2
                    softmax_alpha(b, g, b - gstart)
                if w == 0:
                    emit_x2()
                if b0 + 2 == gstart + gsize:
                    boundary(g)
    _split_excess_waits(nc)
    return nc


def _split_excess_waits(nc: bass.Bass, max_waits: int = 1) -> None:
    """Walrus's per-instruction sync-wait slots are limited; move excess
    on_wait entries onto wait-only NoOps inserted just before the
    instruction (same engine, so ordering is preserved)."""
    for fn in nc.m.functions:
        for blk in fn.blocks:
            new = []
            for inst in blk.instructions:
                si = inst.sync_info
                waits = list(si.on_wait) if si is not None and si.on_wait else []
                if len(waits) > max_waits:
                    extra, keep = waits[:-max_waits], waits[-max_waits:]
                    for ci in range(0, len(extra), max_waits):
                        nop = mybir.InstNoOp(
                            name=f"{inst.name}-wsplit{ci}", ins=[], outs=[]
                        )
                        nop.engine = inst.engine
                        nop.sync_info = mybir.SyncInfo(
                            on_wait=extra[ci : ci + max_waits], on_update=[]
                        )
                        new.append(nop)
                    inst.sync_info = mybir.SyncInfo(
                        on_wait=keep, on_update=list(si.on_update or [])
                    )
                new.append(inst)
            blk.instructions[:] = new


def _split_bf16(a: np.ndarray) -> tuple[np.ndarray, np.ndarray]:
    hi = a.astype(BF16_NP)
    lo = (a - hi.astype(np.float32)).astype(BF16_NP)
    return hi, lo


def _pad_u8(w_w: np.ndarray) -> np.ndarray:
    u = np.zeros((128, 2, 16), np.float32)
    u[:, :, :KT2] = (w_w[0, :H] * USCALE).reshape(KT2, 2, 128).transpose(2, 1, 0)
    return np.ascontiguousarray(u.reshape(128, 2 * 16)).astype(FP8_NP)


def _host_prep(inputs: dict) -> list[dict]:
    hidden = np.asarray(inputs["hidden"], np.float32)
    W_h = np.asarray(inputs["W_h"], np.float32)
    b_h = np.asarray(inputs["b_h"], np.float32)
    w_w = np.asarray(inputs["w_w"], np.float32)
    W_p = np.asarray(inputs["W_p"], np.float32)
    b_p = np.asarray(inputs["b_p"], np.float32)
    W_x = np.asarray(inputs["W_x"], np.float32)
    b_x = np.asarray(inputs["b_x"], np.float32)

    # G selector: row i = 2q + m//64 (q=0 block alone serves 2-row groups)
    selgm = np.zeros((4, 2, 128), np.float32)
    for q in range(2):
        for m in range(128):
            selgm[2 * q + m // 64, q, m] = 1.0

    wxT = np.ascontiguousarray(W_x.T)
    wx_hi, _ = _split_bf16(wxT)
    hlT = np.ascontiguousarray(hidden[:, -1, :].T)
    hl_hi, hl_lo = _split_bf16(hlT)
    bpx_hi, bpx_lo = _split_bf16((b_p + b_x).reshape(1, H))

    shared = {
        "whQ8": np.ascontiguousarray(
            (W_h.T * WSCALE)
            .reshape(KT2, 128, 2, MT, 128)
            .transpose(3, 1, 0, 2, 4)
            .reshape(MT, 128, KT2 * 2 * 128)
        ).astype(FP8_NP),
        "bh": np.ascontiguousarray(b_h.reshape(MT, 128).T),
        "u8": _pad_u8(w_w),
        # wpT8[p, kt2, j, n] = W_p.T[(kt2*2+j)*128 + p, n] * WPS
        "wpT8": np.ascontiguousarray(
            (W_p.T * WPS).reshape(KT2, 2, 128, H).transpose(2, 0, 1, 3).reshape(
                128, KT2 * 2 * H
            )
        ).astype(FP8_NP),
        "wxT_hi": wx_hi,
        "hlastT_hi": hl_hi,
        "hlastT_lo": hl_lo,
        "selg": selgm.reshape(4, 2 * 128).astype(BF16_NP),
        "bpx": np.concatenate([bpx_hi, bpx_lo], axis=1),
        "ones": np.ones((1, B), BF16_NP),
        "ident": np.eye(4, dtype=np.float32).astype(BF16_NP),
    }

    in_maps = []
    for c in range(NCORES):
        slab = hidden[c * PB : (c + 1) * PB]          # [PB, T, H]
        m = dict(shared)
        m["xQ8"] = np.ascontiguousarray(
            slab.reshape(PB, T, KT2, 128, 2)
            .transpose(0, 3, 2, 4, 1)
            .reshape(PB, 128, KT2 * 2 * T)
        ).astype(FP8_NP)
        # ht8[b][p, tt2, j, h] = hidden[b, t = (tt2*2+j)*128 + p, h]
        m["ht8"] = np.ascontiguousarray(
            slab.reshape(PB, TT2, 2, 128, H)
            .transpose(0, 3, 1, 2, 4)
            .reshape(PB, 128, TT2 * 2 * H)
        ).astype(FP8_NP)
        in_maps.append(m)
    return in_maps


def _ensure_ntff_hook() -> None:
    """The agent image's antenv lacks axon_hooks; register a shim module
    wired to the libaxon NTFF profile hook so trace=True works."""
    try:
        from antenv.axon_hooks import get_axon_ntff_profile_hook  # noqa: F401
        return
    except ImportError:
        pass
    import types
    import antenv
    from trn_agent_boot.trn_boot import _ntff_profile_via_ctypes

    mod = types.ModuleType("antenv.axon_hooks")
    holder = {"hook": _ntff_profile_via_ctypes("/opt/axon/libaxon_pjrt.so")}
    mod.get_axon_ntff_profile_hook = lambda: holder["hook"]
    mod.set_axon_ntff_profile_hook = lambda h: holder.__setitem__("hook", h)
    sys.modules["antenv.axon_hooks"] = mod
    antenv.axon_hooks = mod


def run(inputs: dict, trace: bool = False, **kw):
    if trace:
        _ensure_ntff_hook()
    if "nc" not in _CACHE:
        _CACHE["nc"] = _build_nc()
    nc = _CACHE["nc"]
    in_maps = _host_prep(inputs)
    res = run_bass_kernel_spmd(nc, in_maps, list(range(NCORES)), trace=trace, **kw)
    out = np.empty((B, B, H), np.float32)
    for c in range(NCORES):
        out[c * PB : (c + 1) * PB] = np.asarray(res.results[c]["out"], np.float32)
    return out, res


def kernel(**inputs) -> np.ndarray:
    out, _ = run(inputs)
    return out
